# revision 24
# baseline (speedup 1.0000x reference)
"""MoE ExpertRouter kernel for 8x TRN2 NeuronCores (Bass/Tile).

Problem (hardcoded):
  x [8192, 1024] fp32; gate = softmax(relu(x@Wg1+bg1)@Wg2+bg2)  [8192, 8]
  h_e = relu(x@W1[e]+b1[e]); out_e = h_e@W2[e]+b2[e]
  out = sum_e gate[:, e] * out_e   [8192, 1024]

Strategy: data-parallel over tokens. Each of the 8 cores gets 1024 tokens
and computes the gate + all 8 experts for them; host concatenates the
per-core outputs. No collectives.

Host/transfer layer (the wall-clock bottleneck under axon):
  - ALL weights are baked into the program as Const DRAM tensors via
    nc.inline_tensor(): neff_packager embeds them in the NEFF and the
    runtime DMAs them to device HBM once at model-load time. Per-call
    traffic is then just xT (2 MiB fp16 per core) in and out (2 MiB fp16
    per core) back, instead of ~137 MiB of replicated weights per core
    per call through the ~35 MB/s axon tunnel.
  - the compiled+loaded executable is cached in a module global keyed by
    a weight fingerprint, so repeat kernel() calls skip jit/lowering/
    NEFF-load entirely (run_bass_kernel_spmd builds a fresh jax.jit per
    call, which would re-ship the const-laden executable every time; the
    cached runner here is a faithful mirror of its axon code path).
  - zero buffers for ExternalOutputs are materialized on device inside
    the jitted body (jnp.zeros), not shipped from host.

Per-core device kernel layout (unchanged from the tuned baseline):
  - host passes xT = x_shard.T [D=1024, T=1024] so the contraction dim (D)
    lands on SBUF partitions with no on-device transposes.
  - layer 1 (per expert, h-chunk): hT[h, tok] = relu(W1e.T-block matmuls
    vs xT) with per-partition bias via ScalarE activation.
  - layer 2: out[tok, dout] psum accumulation over the h-chunk k-tiles
    (lhsT = hT block, rhs = W2e rows); gate-weighted accumulation into an
    SBUF fp32 accumulator via DVE scalar_tensor_tensor (out += g_e*psum).
  - expert bias b2: out_acc is INITIALIZED to sum_e g_e*b2[e] via a K=8
    matmul (lhsT = transposed gate block, rhs = stacked b2).
  - head: warmup matmuls run during the initial DMA wait so the PE HAM
    clock-gate is already 8/8 (2.4 GHz) when real work lands.
  - all dense matmuls in fp16 (FWL weight loads, full PE rate); PSUM
    accumulation stays fp32. The last expert's gate-weighted accumulate
    writes a fp16 tile that is DMA'd out (halves the D2H fetch).
"""

import hashlib
import os

import numpy as np

import concourse.bass as bass
import concourse.mybir as mybir
import concourse.tile as tile
from concourse import bacc
from concourse import bass2jax
from concourse.bass_utils import run_bass_kernel_spmd

F32 = mybir.dt.float32
F16 = mybir.dt.float16
OUT_F32 = os.environ.get("KERNEL_OUT_F32", "0") == "1"
ODT = F32 if OUT_F32 else F16
ONP = np.float32 if OUT_F32 else np.float16

X_INT8 = os.environ.get("KERNEL_X_INT8", "1") == "1"
# int8 output saved only ~12 ms (D2H is per-shard latency-bound, not
# bandwidth-bound) while raising rel err 8.0e-3 -> 1.14e-2; keep f16 out.
OUT_INT8 = os.environ.get("KERNEL_OUT_INT8", "0") == "1"

D = 1024          # input dim
H = 4096          # expert hidden dim
E = 8             # num experts
N_CORES = 8
N_TOKENS = 8192
P = 128           # SBUF partitions
HK = H // P       # 32 h k-tiles
DK = D // P       # 8 d k-tiles
HC = 8            # h k-tiles per expert chunk
NCHUNK = HK // HC  # 4 expert chunks
DOUT_N = 512      # layer-2 / layer-1 moving free dim
GATE_CHUNKS = [2, 2, 4, 8, 8, 8]   # gate-phase h k-tiles per chunk (sum=HK)
N_WARM = 85       # PE warmup matmuls during the head DMA wait


def build_nc(T, w):
    """Build the single-core Bass program for T tokens.

    ``w`` holds the host-prepped weight arrays (see _prep_shared); they are
    embedded as Const tensors so they ship inside the NEFF, not per call.
    """
    TM = T // P                     # token m-tiles
    tok_slices = []                 # (start, size) moving slices of tokens
    t0 = 0
    while t0 < T:
        sz = min(512, T - t0)
        tok_slices.append((t0, sz))
        t0 += sz

    nc = bacc.Bacc(
        "TRN2", target_bir_lowering=False, debug=False, num_devices=N_CORES
    )
    if X_INT8:
        xT = nc.dram_tensor("xT", [D, T], mybir.dt.int8,
                            kind="ExternalInput").ap()
        xs = nc.dram_tensor("xs", [P, DK], F32, kind="ExternalInput").ap()
    else:
        xT = nc.dram_tensor("xT", [D, T], F16, kind="ExternalInput").ap()
        xs = None
    Wg1 = nc.inline_tensor(w["Wg1"], name="cWg1").ap()
    bg1T = nc.inline_tensor(w["bg1T"], name="cbg1T").ap()
    Wg2T = nc.inline_tensor(w["Wg2T"], name="cWg2T").ap()
    bg2B = nc.inline_tensor(w["bg2B"], name="cbg2B").ap()
    W1 = nc.inline_tensor(w["W1"], name="cW1").ap()
    b1T = nc.inline_tensor(w["b1T"], name="cb1T").ap()
    W2 = nc.inline_tensor(w["W2"], name="cW2").ap()
    b2h = nc.inline_tensor(w["b2h"], name="cb2h").ap()
    identf = nc.inline_tensor(w["identf"], name="cidentf").ap()
    if OUT_INT8:
        # int8 out + per-token scale (token t = m*P + p lives at [p, m])
        out = nc.dram_tensor("out", [T, D], mybir.dt.int8,
                             kind="ExternalOutput").ap()
        out_s = nc.dram_tensor("out_s", [P, TM], F32,
                               kind="ExternalOutput").ap()
    else:
        out = nc.dram_tensor("out", [T, D], ODT, kind="ExternalOutput").ap()
        out_s = None

    with tile.TileContext(nc) as tc:
        _build(nc, tc, T, TM, tok_slices,
               xT, xs, Wg1, bg1T, Wg2T, bg2B, W1, b1T, W2, b2h, identf,
               out, out_s)
    nc.compile()
    return nc


def _build(nc, tc, T, TM, tok_slices,
           xT, xs, Wg1, bg1T, Wg2T, bg2B, W1, b1T, W2, b2h, identf,
           out, out_s):
    ctxs = []

    def pool(name, bufs, space="SBUF"):
        p = tc.tile_pool(name=name, bufs=bufs, space=space)
        ctxs.append(p)
        return p.__enter__()

    persist = pool("persist", 1)
    w1pool = pool("w1pool", 17)
    w2pool = pool("w2pool", 9)
    psum1 = pool("psum1", 3, space="PSUM")
    psum2 = pool("psum2", 4, space="PSUM")
    psumL = pool("psumL", 1, space="PSUM")
    small = pool("small", 4)

    TS0 = tok_slices[0][1]          # first token slice size (512)

    # ---- persistent SBUF tensors ----
    xT_r = xT.rearrange("(k p) t -> p k t", p=P)
    xT_sb = persist.tile([P, DK, T], F16, tag="xT_sb")
    if X_INT8:
        # int8 x lands in xq_sb; DVE dequant (xq * per-channel scale)
        # fills the f16 xT_sb the matmuls consume. ~6us per half, hidden
        # under the head DMA window.
        xq_sb = persist.tile([P, DK, T], mybir.dt.int8, tag="xq_sb")
        xs_sb = persist.tile([P, DK], F32, tag="xs_sb")
        nc.sync.dma_start(out=xs_sb[:], in_=xs[:, :])

    def load_x(a, b):
        """Bring x columns [a:b) into xT_sb (DMA + optional dequant)."""
        if X_INT8:
            nc.sync.dma_start(out=xq_sb[:, :, a:b], in_=xT_r[:, :, a:b])
            for dk in range(DK):
                nc.vector.tensor_scalar_mul(
                    xT_sb[:, dk, a:b], xq_sb[:, dk, a:b],
                    xs_sb[:, dk:dk + 1])
        else:
            nc.sync.dma_start(out=xT_sb[:, :, a:b], in_=xT_r[:, :, a:b])

    # first token-half lands first; second half (if any) issued after
    # chunk-0 weights
    load_x(0, TS0)
    hT = persist.tile([P, HC, T], F16, tag="hT")
    out_acc = persist.tile([P, TM, D], F32, tag="out_acc")
    out_h = persist.tile([P, TM, D], F16 if OUT_INT8 else ODT, tag="out_h")
    if OUT_INT8:
        out_q = persist.tile([P, TM, D], mybir.dt.int8, tag="out_q")
        out_s_sb = persist.tile([P, TM], F32, tag="out_s_sb")
        absb = persist.tile([P, D], F32, tag="absb")
        c_inv127 = persist.tile([P, 1], F32, tag="c_inv127")
        nc.vector.memset(c_inv127[:], 1.0 / 127.0)
        c_eps = persist.tile([P, 1], F32, tag="c_eps")
        nc.vector.memset(c_eps[:], 1e-30)
    gate_sb = persist.tile([P, TM * E], F32, tag="gate_sb")
    logits_sb = persist.tile([P, TM * E], F32, tag="logits_sb")
    bg1_sb = persist.tile([P, HK], F32, tag="bg1_sb")
    nc.sync.dma_start(out=bg1_sb[:], in_=bg1T[:, :])
    wg2_sb = persist.tile([P, HK, E], F16, tag="wg2_sb")
    nc.sync.dma_start(out=wg2_sb[:], in_=Wg2T[:, :, :])
    bg2_sb = persist.tile([P, E], F32, tag="bg2_sb")
    nc.sync.dma_start(out=bg2_sb[:], in_=bg2B[:, :])
    b1_sb = persist.tile([P, E, HK], F32, tag="b1_sb")
    b2_sb = persist.tile([E, D], F16, tag="b2_sb")
    ident_sb = persist.tile([P, P], F32, tag="ident_sb")
    gT = persist.tile([E, T], F16, tag="gT")

    # ---- PE warmup: keep the HAM clock-gate busy during the head DMA ----
    warm = persist.tile([P, P], F16, tag="warm")
    nc.vector.memset(warm[:], 0.0)
    for _ in range(N_WARM):
        pw = psum1.tile([P, P], F32, tag="ps1")
        nc.tensor.matmul(pw[:, :], warm[:, :], warm[:, :], start=True, stop=True)

    def mm(ps, lhsT, rhs, start, stop):
        nc.tensor.matmul(ps, lhsT, rhs, start=start, stop=stop)

    def layer1(wtiles, bias_col, hc):
        """hT[:, hm, :] = relu(sum_dk wtiles[dk][:,hm-block].T @ xT + bias)"""
        for (ts, tsz) in tok_slices:
            for hm in range(hc):
                ps = psum1.tile([P, DOUT_N], F32, tag="ps1")
                for dk in range(DK):
                    mm(ps[:, :tsz],
                       wtiles[dk][:, hm * P:(hm + 1) * P],
                       xT_sb[:, dk, ts:ts + tsz],
                       start=(dk == 0), stop=(dk == DK - 1))
                nc.scalar.activation(
                    hT[:, hm, ts:ts + tsz], ps[:, :tsz],
                    mybir.ActivationFunctionType.Relu,
                    bias=bias_col(hm),
                )

    # ================= gate =================
    hbase = 0
    for ci, hc in enumerate(GATE_CHUNKS):
        wtiles = []
        for dk in range(DK):
            t = w1pool.tile([P, HC * P], F16, tag="w1t")
            nc.sync.dma_start(
                out=t[:, :hc * P],
                in_=Wg1[dk * P:(dk + 1) * P, hbase * P:(hbase + hc) * P])
            wtiles.append(t)
        if ci == 0 and TS0 < T:
            # second xT token-half: after chunk-0 weights, before chunk 1
            load_x(TS0, T)
        elif ci == 1:
            # small tensors first needed in the expert phase — emitted here
            # to keep them out of the head's critical DMA window
            nc.sync.dma_start(out=b1_sb[:], in_=b1T[:, :, :])
            nc.sync.dma_start(out=b2_sb[:], in_=b2h[:, :])
            nc.sync.dma_start(out=ident_sb[:], in_=identf[:, :])
        layer1(wtiles,
               lambda hm, hb=hbase: bg1_sb[:, hb + hm:hb + hm + 1], hc)
        # logits partial: [tok, E] += hT_chunk.T-blocks @ Wg2 rows
        for m in range(TM):
            psL = psumL.tile([P, E], F32, tag="psL")
            for k in range(hc):
                mm(psL[:, :],
                   hT[:, k, m * P:(m + 1) * P],
                   wg2_sb[:, hbase + k, :],
                   start=(k == 0), stop=(k == hc - 1))
            if ci == 0:
                # fold bg2 in once (host pre-broadcast to all partitions)
                nc.vector.tensor_tensor(
                    out=logits_sb[:, m * E:(m + 1) * E],
                    in0=psL[:, :], in1=bg2_sb[:, :],
                    op=mybir.AluOpType.add)
            else:
                nc.vector.tensor_tensor(
                    out=logits_sb[:, m * E:(m + 1) * E],
                    in0=logits_sb[:, m * E:(m + 1) * E],
                    in1=psL[:, :], op=mybir.AluOpType.add)
        hbase += hc

    # softmax over E per token
    for m in range(TM):
        sl = logits_sb[:, m * E:(m + 1) * E]
        mx = small.tile([P, 1], F32, tag="mx")
        nc.vector.tensor_reduce(mx[:], sl, axis=mybir.AxisListType.X,
                                op=mybir.AluOpType.max)
        ex = small.tile([P, E], F32, tag="ex")
        nc.vector.tensor_scalar_sub(ex[:], sl, mx[:])
        nc.scalar.activation(ex[:], ex[:], mybir.ActivationFunctionType.Exp)
        sm = small.tile([P, 1], F32, tag="sm")
        nc.vector.tensor_reduce(sm[:], ex[:], axis=mybir.AxisListType.X,
                                op=mybir.AluOpType.add)
        rc = small.tile([P, 1], F32, tag="rc")
        nc.vector.reciprocal(rc[:], sm[:])
        nc.vector.tensor_scalar_mul(gate_sb[:, m * E:(m + 1) * E], ex[:], rc[:])

    # ================= experts =================
    for e in range(E):
        for c in range(NCHUNK):
            w1tiles = []
            w2tiles = []
            for dk in range(DK):
                t = w1pool.tile([P, HC * P], F16, tag="w1t")
                nc.sync.dma_start(
                    out=t[:],
                    in_=W1[e, dk * P:(dk + 1) * P,
                           c * H // NCHUNK:(c + 1) * H // NCHUNK])
                w1tiles.append(t)
            for k in range(HC):
                t = w2pool.tile([P, D], F16, tag="w2t")
                nc.sync.dma_start(
                    out=t[:], in_=W2[e, (c * HC + k) * P:(c * HC + k + 1) * P, :])
                w2tiles.append(t)

            layer1(w1tiles,
                   lambda hm, e=e, c=c: b1_sb[:, e, c * HC + hm:c * HC + hm + 1],
                   HC)

            if e == 0 and c == 0:
                # out_acc init: sum_e gate[:,e]*b2[e,:] via K=8 matmuls.
                # (Emitted after expert-0 chunk-0 L1 so the PE never waits
                # on the softmax DVE chain.)
                for m in range(TM):
                    psT = psum2.tile([E, P], F32, tag="ps2")
                    nc.tensor.transpose(
                        psT[:, :], gate_sb[:, m * E:(m + 1) * E], ident_sb[:, :])
                    nc.scalar.copy(gT[:, m * P:(m + 1) * P], psT[:, :])
                for m in range(TM):
                    for n in range(D // DOUT_N):
                        psI = psum2.tile([P, DOUT_N], F32, tag="ps2")
                        mm(psI[:, :], gT[:, m * P:(m + 1) * P],
                           b2_sb[:, n * DOUT_N:(n + 1) * DOUT_N],
                           start=True, stop=True)
                        nc.vector.tensor_copy(
                            out_acc[:, m, n * DOUT_N:(n + 1) * DOUT_N], psI[:, :])

            # layer 2: accumulate over the chunk's h k-tiles
            # (only the final chunk of the final expert completes the sum
            # and may write the output staging tile)
            last = (e == E - 1 and c == NCHUNK - 1)
            out_r = out.rearrange("(m p) d -> p m d", p=P)
            for m in range(TM):
                for n in range(D // DOUT_N):
                    ps = psum2.tile([P, DOUT_N], F32, tag="ps2")
                    for k in range(HC):
                        mm(ps[:, :],
                           hT[:, k, m * P:(m + 1) * P],
                           w2tiles[k][:, n * DOUT_N:(n + 1) * DOUT_N],
                           start=(k == 0), stop=(k == HC - 1))
                    g = gate_sb[:, m * E + e:m * E + e + 1]
                    dst = out_h if last else out_acc
                    nc.vector.scalar_tensor_tensor(
                        out=dst[:, m, n * DOUT_N:(n + 1) * DOUT_N],
                        in0=ps[:, :], scalar=g,
                        in1=out_acc[:, m, n * DOUT_N:(n + 1) * DOUT_N],
                        op0=mybir.AluOpType.mult,
                        op1=mybir.AluOpType.add)
                    if last and not OUT_INT8:
                        nc.sync.dma_start(
                            out=out_r[:, m, n * DOUT_N:(n + 1) * DOUT_N],
                            in_=out_h[:, m, n * DOUT_N:(n + 1) * DOUT_N])
                if last and OUT_INT8:
                    # per-token quantize row m: scale = absmax/127
                    nc.scalar.activation(
                        absb[:, :], out_h[:, m, :],
                        mybir.ActivationFunctionType.Abs)
                    amax = small.tile([P, 1], F32, tag="amax")
                    nc.vector.tensor_reduce(
                        amax[:], absb[:, :], axis=mybir.AxisListType.X,
                        op=mybir.AluOpType.max)
                    nc.vector.tensor_tensor(
                        out=amax[:], in0=amax[:], in1=c_eps[:],
                        op=mybir.AluOpType.max)
                    nc.vector.tensor_tensor(
                        out=out_s_sb[:, m:m + 1], in0=amax[:],
                        in1=c_inv127[:], op=mybir.AluOpType.mult)
                    inv = small.tile([P, 1], F32, tag="inv")
                    nc.vector.reciprocal(inv[:], out_s_sb[:, m:m + 1])
                    for n in range(D // DOUT_N):
                        nc.vector.tensor_scalar_mul(
                            out_q[:, m, n * DOUT_N:(n + 1) * DOUT_N],
                            out_h[:, m, n * DOUT_N:(n + 1) * DOUT_N],
                            inv[:])
                    nc.sync.dma_start(out=out_r[:, m, :],
                                      in_=out_q[:, m, :])
            if last and OUT_INT8:
                nc.sync.dma_start(out=out_s[:, :], in_=out_s_sb[:, :])

    for p in reversed(ctxs):
        p.__exit__(None, None, None)


# ---------------- host side ----------------

_STATE = {}
LAST_RESULTS = None


def _prep_shared(Wg1, bg1, Wg2, bg2, W1, b1, W2, b2):
    """Host-side rearrangements shared by all cores."""
    Wg1 = np.ascontiguousarray(np.asarray(Wg1).astype(np.float16))
    bg1 = np.asarray(bg1, dtype=np.float32)
    Wg2 = np.asarray(Wg2).astype(np.float16)
    bg2 = np.asarray(bg2, dtype=np.float32)
    W1 = np.ascontiguousarray(np.asarray(W1).astype(np.float16))
    b1 = np.asarray(b1, dtype=np.float32)
    W2 = np.ascontiguousarray(np.asarray(W2).astype(np.float16))
    b2 = np.asarray(b2, dtype=np.float32)

    bg1T = np.ascontiguousarray(bg1.reshape(HK, P).T)                 # [128, 32]
    Wg2T = np.ascontiguousarray(Wg2.reshape(HK, P, E).transpose(1, 0, 2))  # [128,32,8]
    bg2B = np.ascontiguousarray(np.broadcast_to(bg2[None, :], (P, E)).copy())
    b1T = np.ascontiguousarray(b1.reshape(E, HK, P).transpose(2, 0, 1))    # [128,8,32]
    b2h = np.ascontiguousarray(b2.astype(np.float16))                  # [8, 1024]
    identf = np.eye(P, dtype=np.float32)
    return dict(Wg1=Wg1, bg1T=bg1T, Wg2T=Wg2T, bg2B=bg2B,
                W1=W1, b1T=b1T, W2=W2, b2h=b2h, identf=identf)


def _fingerprint(T, *arrs):
    h = hashlib.sha1()
    h.update(str(T).encode())
    for a in arrs:
        a = np.asarray(a)
        h.update(str(a.shape).encode())
        h.update(str(a.dtype).encode())
        flat = a.reshape(-1)
        step = max(1, flat.size // 1024)
        h.update(np.ascontiguousarray(flat[::step]).tobytes())
    return h.hexdigest()


def _make_runner(nc):
    """Cached-executable mirror of run_bass_kernel_spmd's axon path.

    run_bass_kernel_spmd builds a fresh jax.jit per call, which re-lowers
    and re-loads the (const-laden) executable every time. This builds the
    sharded jit once; repeat calls hit the pjit C++ fast path and only
    move the real inputs/outputs.

    The neuronx_cc hook requires every bass_exec operand to be a jit
    parameter (in order), so the ExternalOutput zero-buffers must be
    parameters too. We pass a persistent device-resident zeros array for
    them WITHOUT donation: libneuronpjrt binds NEFF outputs to the custom
    call result buffers, and this kernel writes every element of `out`,
    so the pre-zeroed staging buffer's contents never matter and it never
    needs re-shipping.
    """
    import jax
    from jax.experimental.shard_map import shard_map
    from jax.sharding import Mesh, NamedSharding, PartitionSpec

    bass2jax.install_neuronx_cc_hook()

    assert nc.dbg_addr is None, "build with debug=False"
    partition_name = (
        nc.partition_id_tensor.name if nc.partition_id_tensor else None
    )

    in_names = []
    in_avals = []
    out_names = []
    out_avals = []
    for alloc in nc.m.functions[0].allocations:
        if not isinstance(alloc, mybir.MemoryLocationSet):
            continue
        name = alloc.memorylocations[0].name
        if alloc.kind == "ExternalInput":
            if name != partition_name:
                in_names.append(name)
                in_avals.append(jax.core.ShapedArray(
                    tuple(alloc.tensor_shape), mybir.dt.np(alloc.dtype)))
        elif alloc.kind == "ExternalOutput":
            out_names.append(name)
            out_avals.append(jax.core.ShapedArray(
                tuple(alloc.tensor_shape), mybir.dt.np(alloc.dtype)))

    all_names = list(in_names) + list(out_names)
    if partition_name is not None:
        all_names.append(partition_name)

    def _body(*args):
        operands = list(args)
        if partition_name is not None:
            operands.append(bass2jax.partition_id_tensor())
        outs = bass2jax._bass_exec_p.bind(
            *operands,
            out_avals=tuple(out_avals),
            in_names=tuple(all_names),
            out_names=tuple(out_names),
            lowering_input_output_aliases=(),
            sim_require_finite=True,
            sim_require_nnan=True,
            nc=nc,
        )
        return tuple(outs)

    devices = jax.devices()[:N_CORES]
    assert len(devices) == N_CORES
    mesh = Mesh(np.asarray(devices), ("core",))
    n_params = len(in_names) + len(out_names)
    in_specs = (PartitionSpec("core"),) * n_params
    out_specs = (PartitionSpec("core"),) * len(out_names)
    mapped = shard_map(_body, mesh=mesh, in_specs=in_specs,
                       out_specs=out_specs, check_rep=False)

    global_avals = [
        jax.ShapeDtypeStruct((N_CORES * a.shape[0], *a.shape[1:]), a.dtype)
        for a in in_avals + out_avals
    ]

    def compile_fn():
        return (jax.jit(mapped, keep_unused=True)
                .lower(*global_avals).compile())

    try:
        compiled = bass2jax.fast_dispatch_compile(compile_fn)
    except Exception:
        compiled = jax.jit(mapped, keep_unused=True)

    # persistent device-resident dummy buffers for the output params
    sharding = NamedSharding(mesh, PartitionSpec("core"))
    dummies = [
        jax.device_put(np.zeros(a.shape, a.dtype), sharding)
        for a in global_avals[len(in_names):]
    ]

    return compiled, in_names, out_names, dummies


def kernel(x, Wg1, bg1, Wg2, bg2, W1, b1, W2, b2):
    global LAST_RESULTS
    x = np.asarray(x, dtype=np.float32)
    n_tok = x.shape[0]
    T = n_tok // N_CORES

    trace = os.environ.get("BASS_KERNEL_TRACE", "0") == "1"
    # token batches pipelined through the tunnel (prep/H2D of batch b+1
    # overlaps execute/D2H of batch b); each batch is its own NEFF launch
    NB = 1 if trace else int(os.environ.get("KERNEL_NB", "1"))
    Tc = T // NB

    fp = _fingerprint(Tc, Wg1, bg1, Wg2, bg2, W1, b1, W2, b2)
    st = _STATE.get(fp)
    if st is None:
        shared = _prep_shared(Wg1, bg1, Wg2, bg2, W1, b1, W2, b2)
        nc = build_nc(Tc, shared)
        st = {"nc": nc}
        _STATE[fp] = st

    xr = x.reshape(N_CORES, NB, Tc, D)

    def prep_batch(b):
        """Per-core inputs for token batch b, in ExternalInput order."""
        xi = xr[:, b]                           # [cores, Tc, D] f32
        if X_INT8:
            s = np.maximum(np.abs(xi).max(axis=1), 1e-30)   # [cores, D]
            q = np.rint(
                xi * (127.0 / s)[:, None, :]).astype(np.int8)
            xb = q.transpose(0, 2, 1).reshape(N_CORES * D, Tc)
            xsb = np.ascontiguousarray(
                (s / 127.0).astype(np.float32)
                .reshape(N_CORES, DK, P).transpose(0, 2, 1)
            ).reshape(N_CORES * P, DK)
            return [xb, xsb]
        xb = (xi.transpose(0, 2, 1)
              .astype(np.float16).reshape(N_CORES * D, Tc))
        return [xb]

    if trace:
        # profiling path: fresh jit inside run_bass_kernel_spmd, NTFF trace
        tmpdir = os.environ.get("BASS_KERNEL_TRACE_DIR")
        ins = prep_batch(0)
        names = ["xT", "xs"] if X_INT8 else ["xT"]
        in_maps = [
            {n: v.reshape(N_CORES, -1, v.shape[-1])[i]
             for n, v in zip(names, ins)}
            for i in range(N_CORES)
        ]
        res = run_bass_kernel_spmd(st["nc"], in_maps, list(range(N_CORES)),
                                   trace=True, tmpdir=tmpdir)
        LAST_RESULTS = res
        outs = []
        for i in range(N_CORES):
            o = res.results[i]["out"].astype(np.float32)
            if OUT_INT8:
                sc = res.results[i]["out_s"]       # [P, TMc]
                o *= sc.T.reshape(-1)[:, None]
            outs.append(o)
        return np.concatenate(outs, axis=0)

    if "runner" not in st:
        st["runner"] = _make_runner(st["nc"])
    compiled, in_names, out_names, dummies = st["runner"]
    oi = out_names.index("out")
    LAST_RESULTS = None

    # dispatch is async: batch b+1's host prep overlaps batch b's flight
    futs = []
    for b in range(NB):
        futs.append(compiled(*prep_batch(b), *dummies))

    out = np.empty((N_CORES, NB, Tc, D), np.float32)
    TMc = Tc // P
    si = out_names.index("out_s") if OUT_INT8 else None
    # start the D2H of all result shards as soon as each is ready
    for f in futs:
        try:
            f[oi].copy_to_host_async()
            if OUT_INT8:
                f[si].copy_to_host_async()
        except AttributeError:
            pass
    for b in range(NB):
        o = np.asarray(futs[b][oi]).reshape(N_CORES, Tc, D)
        if OUT_INT8:
            sc = np.asarray(futs[b][si]).reshape(N_CORES, P, TMc)
            s_tok = sc.transpose(0, 2, 1).reshape(N_CORES, Tc)
            out[:, b] = o.astype(np.float32) * s_tok[:, :, None]
        else:
            out[:, b] = o
    return out.reshape(n_tok, D)
